# revision 34
# baseline (speedup 1.0000x reference)
"""Trainium2 Bass kernel for gated short-time-warp + Conv1d (nn_GW_Conv1D).

Reference computation (per batch element b, C=64 channels, T=32768):
  g = tanh(einsum('ct,c->t', x, est_w)) * 0.5            # velocity, |g| <= 0.5
  d = flow(g)    per 256-window (scaling & squaring, 4 iters), |d| <= 0.5
  xw = interp1d(x, p + d)   per window                    # forward warp
  y = conv1d(xw, conv_w, conv_b, k=3, SAME)               # channel mixing
  d_inv = flow(-g); out = interp1d(y, p + d_inv)          # inverse warp

|d| < 1 always, so each warp is a 3-term elementwise stencil:
  out = x*(1-dn-dp) + x[-1]*dn + x[+1]*dp,  dn=relu(-d), dp=relu(d)
with dn zeroed at window starts and dp zeroed at window ends (the clip).
The edge zeroing is folded into d itself before broadcasting:
  d[win col 0] <- max(d, 0)   (kills dn only; dp unchanged)
  d[win col W-1] <- min(d, 0) (kills dp only; dn unchanged)

v5 architecture (single streaming pass, fp16 datapath):
  - "halves" layout everywhere: partition p = (h, c), free dim = t in half.
  - x loaded once with fp32->fp16 SWDGE cast DMAs (2 per half, h0/h1 pairs
    land on disjoint even/odd SDMA engine sets and run concurrently).
  - einsum as fp16 K=128 matmuls chasing the load.
  - flow per direction on [128 windows, 256] fp16 tiles, all-DVE.
  - per-window warp coefficients: derive dn/dp small, flatten to per-
    (dir,base) rows, replicate each row to 8 contiguous copies (24
    independent DMAs per direction, once), then per chunk a 10-DMA
    depth-4 doubling tree broadcasts to [128, 2*CH] (fwd on sync ring,
    inv on scalar ring; stores on the gpsimd ring to avoid FIFO
    head-of-line blocking).
  - 6-op warp entirely on DVE fp16 2x: s1=(x[-1]-x)*dn; s2=(x[+1]-x)*dp;
    out = x+s1+s2.
  - conv as block-diagonal K=128 fp16 matmuls, bias fused in the
    scalar-engine PSUM evacuation.
  - output stored fp16 (host casts to fp32).

Sharding: pure data parallelism, batch b -> core b (8 cores).
"""
import sys

sys.path.insert(0, "/opt/trn_rl_repo")

import numpy as np
from contextlib import ExitStack

import concourse.bass as bass
import concourse.tile as tile
from concourse import bacc, mybir
from concourse.bass_interp import get_hw_module
from concourse import bass_utils

F32 = mybir.dt.float32
F16 = mybir.dt.float16
AF = mybir.ActivationFunctionType
ALU = mybir.AluOpType

NCORES = 8
C, T, W = 64, 32768, 256
H = T // 2            # 16384 cols per half
FLOW_ITERS = 4
CH = 4096             # main-loop chunk (window-aligned)
NCH = H // CH         # 4 chunks


def _flow_dir(nc, pool, d, sfx, r0, r1):
    """Scaling-and-squaring on rows [r0:r1] of direction tile d (128, 256)
    fp16, all-DVE."""
    R = (r0, r1)
    for _ in range(FLOW_ITERS):
        dn = pool.tile([128, 256], F16, tag="fl_dn" + sfx)
        dp = pool.tile([128, 256], F16, tag="fl_dp" + sfx)
        nc.vector.tensor_scalar(dn[r0:r1], d[r0:r1], -1.0, 0.0, ALU.mult, ALU.max)
        nc.vector.tensor_scalar_max(dp[r0:r1], d[r0:r1], 0.0)
        nc.vector.tensor_scalar_mul(dn[r0:r1, 0:1], dn[r0:r1, 0:1], 0.0)
        nc.vector.tensor_scalar_mul(dp[r0:r1, 255:256], dp[r0:r1, 255:256], 0.0)
        am = pool.tile([128, 256], F16, tag="fl_am" + sfx)
        nc.vector.tensor_tensor(am[r0:r1], dn[r0:r1], dp[r0:r1], ALU.add)
        nc.vector.tensor_scalar(am[r0:r1], am[r0:r1], -1.0, 1.0, ALU.mult, ALU.add)
        itp = pool.tile([128, 256], F16, tag="fl_it" + sfx)
        tmp = pool.tile([128, 256], F16, tag="fl_tm" + sfx)
        nc.vector.tensor_tensor(itp[r0:r1], d[r0:r1], am[r0:r1], ALU.mult)
        nc.vector.tensor_tensor(tmp[r0:r1, 1:256], d[r0:r1, 0:255],
                                dn[r0:r1, 1:256], ALU.mult)
        nc.vector.tensor_tensor(itp[r0:r1, 1:256], itp[r0:r1, 1:256],
                                tmp[r0:r1, 1:256], ALU.add)
        nc.vector.tensor_tensor(tmp[r0:r1, 0:255], d[r0:r1, 1:256],
                                dp[r0:r1, 0:255], ALU.mult)
        nc.vector.tensor_tensor(itp[r0:r1, 0:255], itp[r0:r1, 0:255],
                                tmp[r0:r1, 0:255], ALU.add)
        nc.vector.tensor_tensor(d[r0:r1], d[r0:r1], itp[r0:r1], ALU.add)
    # fold the window-edge clip into d itself
    nc.vector.tensor_scalar_max(d[r0:r1, 0:1], d[r0:r1, 0:1], 0.0)
    nc.vector.tensor_scalar_min(d[r0:r1, 255:256], d[r0:r1, 255:256], 0.0)


def _build_module():
    nc = bacc.Bacc("TRN2", target_bir_lowering=False, debug=False,
                   enable_asserts=False, num_devices=NCORES)
    x = nc.dram_tensor("x", (C, T), F32, kind="ExternalInput").ap()
    ew = nc.dram_tensor("ew", (128, 2), F16, kind="ExternalInput").ap()
    cw = nc.dram_tensor("cw", (128, 384), F16, kind="ExternalInput").ap()
    cb = nc.dram_tensor("cb", (128, 1), F32, kind="ExternalInput").ap()
    y = nc.dram_tensor("y", (C, T), F16, kind="ExternalOutput").ap()

    # per-half views: [64, H] slices of the (C, T) tensors (outer dim 64
    # keeps DMA descriptors striped across many SDMA engines)
    x_h = [x[:, 0:H], x[:, H:T]]
    y_h = [y[:, 0:H], y[:, H:T]]

    with tile.TileContext(nc) as tc, ExitStack() as ctx:
        big = ctx.enter_context(tc.tile_pool(name="big", bufs=1))
        sm = ctx.enter_context(tc.tile_pool(name="sm", bufs=1))

        # persistent tiles: x (halo'd), warped x (halo'd), flat coef rows
        x16 = big.tile([128, H + 2], F16)
        xw = big.tile([128, H + 2], F16)
        # flat coef rows: r = dir*64 + (h*2+hh)*16 + rep*2 + ci
        # (8 replicated (dn,dp) row pairs per (dir,h,hh)), cols = H/2 span hh
        flat = big.tile([128, H // 2], F16)
        nc.gpsimd.memset(x16[:, 0:1], 0.0)
        nc.gpsimd.memset(x16[:, H + 1:H + 2], 0.0)
        nc.gpsimd.memset(xw[:, 0:1], 0.0)
        nc.gpsimd.memset(xw[:, H + 1:H + 2], 0.0)

        ew_sb = sm.tile([128, 2], F16, tag="ew")
        nc.sync.dma_start(ew_sb[:], ew)
        cw_sb = sm.tile([128, 384], F16, tag="cw")
        nc.sync.dma_start(cw_sb[:], cw)
        cb_sb = sm.tile([128, 1], F32, tag="cb")
        nc.sync.dma_start(cb_sb[:], cb)

        # ------- Stage A: cast-load x, fp16 einsum chasing ------------------
        # h0/h1 DMA pairs hit disjoint (even/odd) SDMA engine sets
        for i in (H // 2, 0):
            for h in (0, 1):
                nc.gpsimd.dma_start(x16[h * 64:(h + 1) * 64, 1 + i:1 + i + H // 2],
                                    x_h[h][:, i:i + H // 2])
        g_w = sm.tile([128, 256], F16, tag="gw")         # windows on partitions
        stageA = ctx.enter_context(tc.tile_pool(name="stA", bufs=2))
        with tc.tile_pool(name="psA", bufs=2, space="PSUM") as psA:
            for i in list(range(H // 2, H, 2048)) + list(range(0, H // 2, 2048)):
                pg = psA.tile([2, 2048], F32, tag="pg")
                for j in range(4):
                    nc.tensor.matmul(pg[:, j * 512:(j + 1) * 512], ew_sb[:],
                                     x16[:, 1 + i + j * 512:1 + i + (j + 1) * 512],
                                     start=True, stop=True)
                ge = stageA.tile([2, 2048], F16, tag="ge")
                nc.scalar.copy(ge[:], pg[:])
                fl0 = i // 256
                q, idx = fl0 // 16, fl0 % 16
                for h in (0, 1):
                    r = q * 32 + h * 16 + idx
                    nc.sync.dma_start(g_w[r:r + 8, :], ge[h:h + 1, :])

        # ------- Stage B: tanh, per-direction flow/derive/flatten/replicate -
        g_th = sm.tile([128, 256], F32, tag="gth")
        nc.scalar.activation(g_th[:], g_w[:], AF.Tanh)

        dtiles = {}

        def derive_flatten(dir_, hh, ring):
            if dir_ not in dtiles:
                dtiles[dir_] = sm.tile([128, 256], F16, tag=f"d{dir_}",
                                       name=f"d{dir_}")
            d = dtiles[dir_]
            r0, r1 = 64 * hh, 64 * hh + 64
            nc.vector.tensor_scalar_mul(d[r0:r1], g_th[r0:r1],
                                        0.5 / 16.0 if dir_ == 0 else -0.5 / 16.0)
            _flow_dir(nc, sm, d, str(dir_), r0, r1)
            dn_s = sm.tile([128, 256], F16, tag=f"dn{dir_}")
            dp_s = sm.tile([128, 256], F16, tag=f"dp{dir_}")
            nc.vector.tensor_scalar(dn_s[r0:r1], d[r0:r1], -1.0, 0.0,
                                    ALU.mult, ALU.max)
            nc.vector.tensor_scalar_max(dp_s[r0:r1], d[r0:r1], 0.0)
            for ci_, coef_s in ((0, dn_s), (1, dp_s)):
                for h in (0, 1):
                    fr = dir_ * 64 + (h * 2 + hh) * 16 + ci_
                    for sub in (0, 1):
                        q = 2 * hh + sub
                        ring.dma_start(
                            flat[fr:fr + 1, sub * 4096:(sub + 1) * 4096],
                            coef_s[q * 32 + h * 16:q * 32 + h * 16 + 16, :])
            # replicate each (dn,dp) row pair to 8 copies, wave-major
            for p in (2, 4, 8):
                for h in (0, 1):
                    fr = dir_ * 64 + (h * 2 + hh) * 16
                    ring.dma_start(flat[fr + p:fr + 2 * p, :],
                                   flat[fr:fr + p, :])

        # ------- main streaming loop ----------------------------------------
        cf_pool = ctx.enter_context(tc.tile_pool(name="cf", bufs=4))
        wrk = ctx.enter_context(tc.tile_pool(name="wrk", bufs=1))
        ypool = ctx.enter_context(tc.tile_pool(name="yp", bufs=2))
        psB = ctx.enter_context(tc.tile_pool(name="psB", bufs=4, space="PSUM"))

        def build_coef(ring, dir_, k):
            """Broadcast coef rows for chunk k to a [128, 2*CH] tile
            ([dn | dp]) via a depth-4 doubling tree off 8-replica rows."""
            ct = cf_pool.tile([128, 2 * CH], F16, tag="ct",
                              name=f"ct{dir_}_{k}")
            i0 = k * CH
            hh, off = i0 // (H // 2), i0 % (H // 2)
            for h in (0, 1):
                r0 = dir_ * 64 + (h * 2 + hh) * 16
                ring.dma_start(ct[h * 64:h * 64 + 8, :],
                               flat[r0:r0 + 16, off:off + CH])
            for p in (8, 16, 32):
                for b in (0, 64):
                    ring.dma_start(ct[b + p:b + 2 * p, :], ct[b:b + p, :])
            return ct

        def ct_slices(ct, k):
            return ct[:, 0:CH], ct[:, CH:2 * CH]

        def warp(src, s_off, dn_ap, dp_ap, dst, d_off, n):
            """dst = src + dn*(src[-1]-src) + dp*(src[+1]-src) over n cols."""
            s1 = wrk.tile([128, CH], F16, tag="s1")
            s2 = wrk.tile([128, CH], F16, tag="s2")
            nc.vector.tensor_tensor(s1[:, 0:n], src[:, s_off - 1:s_off - 1 + n],
                                    src[:, s_off:s_off + n], ALU.subtract)
            nc.vector.tensor_tensor(s1[:, 0:n], s1[:, 0:n], dn_ap, ALU.mult)
            nc.vector.tensor_tensor(s2[:, 0:n], src[:, s_off + 1:s_off + 1 + n],
                                    src[:, s_off:s_off + n], ALU.subtract)
            nc.vector.tensor_tensor(s2[:, 0:n], s2[:, 0:n], dp_ap, ALU.mult)
            nc.vector.tensor_tensor(dst[:, d_off:d_off + n],
                                    src[:, s_off:s_off + n], s1[:, 0:n], ALU.add)
            nc.vector.tensor_tensor(dst[:, d_off:d_off + n],
                                    dst[:, d_off:d_off + n], s2[:, 0:n], ALU.add)

        # piece B (chunks 2,3) of the fwd flow first: its coef chains start
        # while the first-half einsum and remaining flows still run
        cts, cis = {}, {}
        derive_flatten(0, 1, nc.sync)
        cts[3] = build_coef(nc.sync, 0, 3)
        cts[2] = build_coef(nc.scalar, 0, 2)
        derive_flatten(0, 0, nc.sync)
        cts[1] = build_coef(nc.sync, 0, 1)
        cts[0] = build_coef(nc.scalar, 0, 0)
        derive_flatten(1, 1, nc.gpsimd)
        cis[2] = build_coef(nc.gpsimd, 1, 2)
        cis[3] = build_coef(nc.gpsimd, 1, 3)
        derive_flatten(1, 0, nc.gpsimd)
        cis[1] = build_coef(nc.gpsimd, 1, 1)
        cis[0] = build_coef(nc.gpsimd, 1, 0)

        # forward warps, ordered so the cross-half conv seam is ready early
        for k in (3, 2, 1, 0):
            i0 = k * CH
            dn_ap, dp_ap = ct_slices(cts[k], k)
            warp(x16, 1 + i0, dn_ap, dp_ap, xw, 1 + i0, CH)
            if k == 3:   # half1's left conv halo = last warped col of half0
                nc.sync.dma_start(xw[64:128, 0:1], xw[0:64, H:H + 1])
            if k == 0:   # half0's right conv halo = first warped col of half1
                nc.sync.dma_start(xw[0:64, H + 1:H + 2], xw[64:128, 1:2])

        # conv + inverse warp + store, per chunk
        for k in (2, 3, 1, 0):
            i0 = k * CH
            y16 = ypool.tile([128, CH + 2], F16, tag="y16")
            nc.vector.memset(y16[:, 0:1], 0.0)
            nc.vector.memset(y16[:, CH + 1:CH + 2], 0.0)
            for b in range(CH // 512):
                pc = psB.tile([128, 512], F32, tag="pc")
                for j in range(3):
                    nc.tensor.matmul(pc[:], cw_sb[:, j * 128:(j + 1) * 128],
                                     xw[:, i0 + b * 512 + j:i0 + b * 512 + j + 512],
                                     start=(j == 0), stop=(j == 2))
                nc.scalar.activation(y16[:, 1 + b * 512:1 + (b + 1) * 512], pc[:],
                                     AF.Identity, bias=cb_sb[:])
            dn_ap, dp_ap = ct_slices(cis[k], k)
            warp(y16, 1, dn_ap, dp_ap, y16, 1, CH)   # in-place
            for h in (0, 1):
                nc.sync.dma_start(y_h[h][:, i0:i0 + CH],
                                  y16[h * 64:(h + 1) * 64, 1:1 + CH])

    nc.compile()
    return nc


def _host_params(est_w, conv_w, conv_b):
    ew = np.zeros((128, 2), np.float16)
    ew[:64, 0] = est_w
    ew[64:, 1] = est_w
    cw = np.zeros((128, 384), np.float16)
    for j in range(3):
        blk = conv_w[:, :, j].T.astype(np.float16)   # (in, out)
        cw[0:64, j * 128:j * 128 + 64] = blk
        cw[64:128, j * 128 + 64:j * 128 + 128] = blk
    cb = np.concatenate([conv_b, conv_b]).astype(np.float32)[:, None]
    return ew, cw, cb


_COMPILED = None


def _get_compiled():
    global _COMPILED
    if _COMPILED is None:
        nc = _build_module()
        nc.m = get_hw_module(nc.m)
        _COMPILED = nc
    return _COMPILED


def kernel(signal, est_w, conv_w, conv_b, _trace=False, _trace_kwargs=None):
    nc = _get_compiled()
    ew, cw, cb = _host_params(np.asarray(est_w, np.float32),
                              np.asarray(conv_w, np.float32),
                              np.asarray(conv_b, np.float32))
    signal = np.ascontiguousarray(np.asarray(signal, np.float32))
    in_maps = [{"x": signal[b], "ew": ew, "cw": cw, "cb": cb}
               for b in range(NCORES)]
    res = bass_utils.run_bass_kernel_spmd(
        nc, in_maps, core_ids=list(range(NCORES)), trace=_trace,
        **(_trace_kwargs or {}))
    out = np.stack([np.asarray(r["y"], np.float32) for r in res.results], axis=0)
    if _trace:
        return out, res
    return out
